# revision 1
# baseline (speedup 1.0000x reference)
"""Trainium2 Bass kernel for nn_JointLearningModel (coref-style joint model).

Sharding: the 384x384 pair grid is split by rows across 8 NeuronCores
(48 rows each). all_mention_representations are replicated (each core
gathers them itself via indirect DMA); params replicated; the scalar
loss is computed per-core over its row block (+ its slice of the
character CE) and summed on the host.
"""

import numpy as np
import ml_dtypes

import concourse.bass as bass
import concourse.mybir as mybir
import concourse.tile as tile
from concourse import bacc
from concourse.bass_utils import run_bass_kernel_spmd

F32 = mybir.dt.float32
BF16 = mybir.dt.bfloat16
I32 = mybir.dt.int32
AF = mybir.ActivationFunctionType
OP = mybir.AluOpType

B, L, H, M = 8, 512, 768, 383
N = M + 1          # 384 rows/cols of the pair grid
NC_ = 8            # cores
R = N // NC_       # 48 rows per core
HC = H // 128      # 6 k-chunks of the hidden dim
NEG = -10000.0
NSRC = B * L + 400 + 1 + 1   # seq rows + speaker rows + dummy + zeros row
DUMMY_ROW = B * L + 400
ZERO_ROW = DUMMY_ROW + 1

_CACHE = {}
LAST_RESULT = None


def _build_program():
    nc = bacc.Bacc(
        "TRN2", target_bir_lowering=False, debug=False, enable_asserts=False
    )

    def din(name, shape, dt):
        return nc.dram_tensor(name, list(shape), dt, kind="ExternalInput")

    # gather source + offset tables
    gsrc = din("gsrc", [NSRC, H], BF16)
    gidx = din("gidx", [128, 3, 3], I32)      # [p, tile, {start,end,spk}]
    gidxl = din("gidxl", [R, 3], I32)         # local rows (per-core)
    ident = din("ident", [128, 128], BF16)
    # pair MLP weights
    waT = din("waT", [128, HC, H], BF16)      # waT[p,ci,o] = Wa.T[ci*128+p, o]
    wbT = din("wbT", [128, HC, H], BF16)
    w2T = din("w2T", [128, HC, H // 2], BF16)
    w3c = din("w3c", [128, 3], BF16)
    b1c = din("b1c", [128, HC], F32)
    b2c = din("b2c", [128, 3], F32)
    # mention-score MLP
    wm1T = din("wm1T", [128, HC, H // 2], BF16)
    bm1c = din("bm1c", [128, 3], F32)
    wm2T = din("wm2T", [128, 3, H // 4], BF16)
    bm2c = din("bm2c", [128, 2], F32)
    wm3c = din("wm3c", [128, 2], BF16)
    # character head
    wc1T = din("wc1T", [128, HC, H // 2], BF16)
    bc1c = din("bc1c", [128, 3], F32)
    wc2T = din("wc2T", [128, 3, 18], BF16)
    bc2r = din("bc2r", [1, 18], F32)
    # per-core loss plumbing
    maskb = din("maskb", [R, N], F32)
    multb = din("multb", [R, N], F32)
    wnll = din("wnll", [R, 1], F32)
    oneh = din("oneh", [R, 18], F32)
    wch = din("wch", [R, 1], F32)

    loss = nc.dram_tensor("loss", [1, 1], F32, kind="ExternalOutput")

    with tile.TileContext(nc) as tc:
        with tc.tile_pool(name="const", bufs=1) as cp:
            # ---- resident tiles (DMA'd once) ----
            def load(name, h):
                t = cp.tile(list(h.shape), h.dtype, name=name)
                nc.sync.dma_start(out=t[:], in_=h.ap())
                return t

            ident_sb = load("ident_sb", ident)
            gidx_sb = load("gidx_sb", gidx)
            gidxl_sb = load("gidxl_sb", gidxl)
            waT_sb = load("waT_sb", waT)
            wbT_sb = load("wbT_sb", wbT)
            w2T_sb = load("w2T_sb", w2T)
            w3c_sb = load("w3c_sb", w3c)
            b1c_sb = load("b1c_sb", b1c)
            b2c_sb = load("b2c_sb", b2c)
            wm1T_sb = load("wm1T_sb", wm1T)
            bm1c_sb = load("bm1c_sb", bm1c)
            wm2T_sb = load("wm2T_sb", wm2T)
            bm2c_sb = load("bm2c_sb", bm2c)
            wm3c_sb = load("wm3c_sb", wm3c)
            wc1T_sb = load("wc1T_sb", wc1T)
            bc1c_sb = load("bc1c_sb", bc1c)
            wc2T_sb = load("wc2T_sb", wc2T)
            bc2r_sb = load("bc2r_sb", bc2r)
            maskb_sb = load("maskb_sb", maskb)
            multb_sb = load("multb_sb", multb)
            wnll_sb = load("wnll_sb", wnll)
            oneh_sb = load("oneh_sb", oneh)
            wch_sb = load("wch_sb", wch)

            one1 = cp.tile([1, R], F32)
            nc.vector.memset(one1[:], 1.0)

            # outputs of the preamble, used by the main loop / epilogue
            at_sb = cp.tile([128, HC, N], BF16)    # A.T   (bf16)
            bb_sb = cp.tile([128, HC, R], F32)     # Bm.T + b1, local rows
            rT = cp.tile([128, HC, N], BF16)       # all_reps.T
            rTl = cp.tile([128, HC, R], BF16)      # local all_reps.T
            mskms = cp.tile([R, N], F32)           # mask + ms[j] broadcast
            sblkf = cp.tile([1, R * N], F32)       # pair scores, flat on part 0

            # ---------- gather mention representations ----------
            with tc.tile_pool(name="gat", bufs=2) as gp:
                reps = []
                for t in range(3):
                    g3 = []
                    for s in range(3):
                        g = gp.tile([128, H], BF16, tag=f"g{s}", name=f"g_{t}_{s}")
                        nc.gpsimd.indirect_dma_start(
                            out=g[:],
                            out_offset=None,
                            in_=gsrc.ap(),
                            in_offset=bass.IndirectOffsetOnAxis(
                                ap=gidx_sb[:, t, s : s + 1], axis=0
                            ),
                        )
                        g3.append(g)
                    rep = cp.tile([128, H], BF16, tag="rep", name=f"rep_{t}")
                    nc.vector.tensor_tensor(
                        out=rep[:], in0=g3[0][:], in1=g3[1][:], op=OP.add
                    )
                    nc.vector.tensor_tensor(
                        out=rep[:], in0=rep[:], in1=g3[2][:], op=OP.add
                    )
                    reps.append(rep)
                # local rows
                gl3 = []
                for s in range(3):
                    gl = gp.tile([R, H], BF16, tag=f"gl{s}", name=f"gl_{s}")
                    nc.gpsimd.indirect_dma_start(
                        out=gl[:],
                        out_offset=None,
                        in_=gsrc.ap(),
                        in_offset=bass.IndirectOffsetOnAxis(
                            ap=gidxl_sb[:, s : s + 1], axis=0
                        ),
                    )
                    gl3.append(gl)
                repl = cp.tile([R, H], BF16)
                nc.vector.tensor_tensor(
                    out=repl[:], in0=gl3[0][:], in1=gl3[1][:], op=OP.add
                )
                nc.vector.tensor_tensor(
                    out=repl[:], in0=repl[:], in1=gl3[2][:], op=OP.add
                )

                # ---------- transpose to [H, mention] layout ----------
                with tc.tile_pool(name="tp_ps", bufs=4, space="PSUM") as tpp:
                    for t in range(3):
                        for c in range(HC):
                            pt = tpp.tile([128, 128], BF16, tag="tp", name=f"pt_{t}_{c}")
                            nc.tensor.transpose(
                                out=pt[:],
                                in_=reps[t][:, c * 128 : (c + 1) * 128],
                                identity=ident_sb[:],
                            )
                            nc.vector.tensor_copy(
                                out=rT[:, c, t * 128 : (t + 1) * 128], in_=pt[:]
                            )
                    for c in range(HC):
                        pt = tpp.tile([128, 128], BF16, tag="tp", name=f"ptl_{c}")
                        nc.tensor.transpose(
                            out=pt[:],
                            in_=repl[:, c * 128 : (c + 1) * 128],
                            identity=ident_sb[:R, :],
                        )
                        nc.vector.tensor_copy(out=rTl[:, c, :], in_=pt[:, :R])

            # ---------- preamble matmuls: A.T, Bb, ms, mask+ms ----------
            with tc.tile_pool(name="pre_ps", bufs=2, space="PSUM") as pp:
                for co in range(HC):
                    pa = pp.tile([128, N], F32, tag="big", name=f"pa_{co}")
                    for ci in range(HC):
                        nc.tensor.matmul(
                            out=pa[:],
                            lhsT=waT_sb[:, ci, co * 128 : (co + 1) * 128],
                            rhs=rT[:, ci, :],
                            start=(ci == 0),
                            stop=(ci == HC - 1),
                        )
                    nc.scalar.copy(out=at_sb[:, co, :], in_=pa[:])
                for co in range(HC):
                    pb = pp.tile([128, R], F32, tag="small", name=f"pb_{co}")
                    for ci in range(HC):
                        nc.tensor.matmul(
                            out=pb[:],
                            lhsT=wbT_sb[:, ci, co * 128 : (co + 1) * 128],
                            rhs=rTl[:, ci, :],
                            start=(ci == 0),
                            stop=(ci == HC - 1),
                        )
                    nc.vector.tensor_scalar(
                        out=bb_sb[:, co, :],
                        in0=pb[:],
                        scalar1=b1c_sb[:, co : co + 1],
                        scalar2=None,
                        op0=OP.add,
                    )
                # mention score MLP (768 -> 384 -> 192 -> 1)
                ms1 = cp.tile([128, 3, N], BF16)
                for co in range(3):
                    pm = pp.tile([128, N], F32, tag="big", name=f"pm_{co}")
                    for ci in range(HC):
                        nc.tensor.matmul(
                            out=pm[:],
                            lhsT=wm1T_sb[:, ci, co * 128 : (co + 1) * 128],
                            rhs=rT[:, ci, :],
                            start=(ci == 0),
                            stop=(ci == HC - 1),
                        )
                    nc.scalar.activation(
                        out=ms1[:, co, :],
                        in_=pm[:],
                        func=AF.Relu,
                        bias=bm1c_sb[:, co : co + 1],
                    )
                ms2 = cp.tile([128, 2, N], BF16)
                for co, sz in enumerate((128, 64)):
                    pm2 = pp.tile([128, N], F32, tag="big", name=f"pm2_{co}")
                    for ci in range(3):
                        nc.tensor.matmul(
                            out=pm2[:sz, :],
                            lhsT=wm2T_sb[:, ci, co * 128 : co * 128 + sz],
                            rhs=ms1[:, ci, :],
                            start=(ci == 0),
                            stop=(ci == 2),
                        )
                    nc.scalar.activation(
                        out=ms2[:sz, co, :],
                        in_=pm2[:sz, :],
                        func=AF.Relu,
                        bias=bm2c_sb[:sz, co : co + 1],
                    )
                pms = pp.tile([1, N], F32, tag="small")
                nc.tensor.matmul(
                    out=pms[:], lhsT=wm3c_sb[:, 0:1], rhs=ms2[:, 0, :],
                    start=True, stop=False,
                )
                nc.tensor.matmul(
                    out=pms[:], lhsT=wm3c_sb[:64, 1:2], rhs=ms2[:64, 1, :],
                    start=False, stop=True,
                )
                ms_sb = cp.tile([1, N], F32)
                nc.vector.tensor_copy(out=ms_sb[:], in_=pms[:])
                # broadcast ms over the 48 rows and add the causal mask
                pbc = pp.tile([R, N], F32, tag="big")
                nc.tensor.matmul(
                    out=pbc[:], lhsT=one1[:], rhs=ms_sb[:], start=True, stop=True
                )
                nc.vector.tensor_tensor(
                    out=mskms[:], in0=pbc[:], in1=maskb_sb[:], op=OP.add
                )

            # ---------- main loop: 48 rows of the pair grid ----------
            with (
                tc.tile_pool(name="lp_sb", bufs=2) as lsb,
                tc.tile_pool(name="lp_ps", bufs=2, space="PSUM") as lps,
                tc.tile_pool(name="sr_ps", bufs=2, space="PSUM") as sps,
            ):
                for r in range(R):
                    h1 = lsb.tile(
                        [128, HC, N], BF16, tag="h1", name=f"h1_{r}", bufs=3
                    )
                    for c in range(HC):
                        nc.vector.tensor_scalar(
                            out=h1[:, c, :],
                            in0=at_sb[:, c, :],
                            scalar1=bb_sb[:, c, r : r + 1],
                            scalar2=0.0,
                            op0=OP.add,
                            op1=OP.max,
                        )
                    h2s = []
                    for hb in range(3):
                        ph = lps.tile(
                            [128, N], F32, tag=f"h2_{hb}", name=f"ph_{r}_{hb}"
                        )
                        for c in range(HC):
                            nc.tensor.matmul(
                                out=ph[:],
                                lhsT=w2T_sb[:, c, hb * 128 : (hb + 1) * 128],
                                rhs=h1[:, c, :],
                                start=(c == 0),
                                stop=(c == HC - 1),
                            )
                        hs = lsb.tile(
                            [128, N], BF16, tag=f"h2s_{hb}", name=f"hs_{r}_{hb}"
                        )
                        nc.scalar.activation(
                            out=hs[:], in_=ph[:], func=AF.Relu,
                            bias=b2c_sb[:, hb : hb + 1],
                        )
                        h2s.append(hs)
                    sr = sps.tile([1, N], F32, tag="srow", name=f"sr_{r}")
                    for hb in range(3):
                        nc.tensor.matmul(
                            out=sr[:], lhsT=w3c_sb[:, hb : hb + 1], rhs=h2s[hb][:],
                            start=(hb == 0), stop=(hb == 2),
                        )
                    nc.vector.tensor_copy(
                        out=sblkf[:, r * N : (r + 1) * N], in_=sr[:]
                    )

            # ---------- epilogue: masked row-softmax loss + char CE ----------
            with (
                tc.tile_pool(name="ep_sb", bufs=1) as ep,
                tc.tile_pool(name="ep_ps", bufs=2, space="PSUM") as eps,
            ):
                sblk = ep.tile([R, N], F32)
                nc.sync.dma_start(out=sblk[:], in_=sblkf[:])
                x = ep.tile([R, N], F32)
                nc.vector.tensor_tensor(out=x[:], in0=sblk[:], in1=mskms[:], op=OP.add)
                rm = ep.tile([R, 1], F32)
                nc.vector.tensor_reduce(
                    out=rm[:], in_=x[:], axis=mybir.AxisListType.X, op=OP.max
                )
                nrm = ep.tile([R, 1], F32)
                nc.vector.tensor_scalar_mul(nrm[:], rm[:], -1.0)
                pexp = ep.tile([R, N], F32)
                z = ep.tile([R, 1], F32)
                nc.scalar.activation(
                    out=pexp[:], in_=x[:], func=AF.Exp, bias=nrm[:, 0:1],
                    accum_out=z[:],
                )
                escr = ep.tile([R, N], F32)
                nc.vector.tensor_tensor(
                    out=escr[:], in0=pexp[:], in1=multb_sb[:], op=OP.mult
                )
                e = ep.tile([R, 1], F32)
                nc.vector.tensor_reduce(
                    out=e[:], in_=escr[:], axis=mybir.AxisListType.X, op=OP.add
                )
                lz = ep.tile([R, 1], F32)
                nc.scalar.activation(out=lz[:], in_=z[:], func=AF.Ln)
                le = ep.tile([R, 1], F32)
                nc.scalar.activation(out=le[:], in_=e[:], func=AF.Ln)
                tnll = ep.tile([R, 1], F32)
                nc.vector.tensor_tensor(
                    out=tnll[:], in0=lz[:], in1=le[:], op=OP.subtract
                )
                pl = eps.tile([1, 1], F32, tag="loss", bufs=1)
                nc.tensor.matmul(
                    out=pl[:], lhsT=tnll[:, 0:1], rhs=wnll_sb[:], start=True,
                    stop=False,
                )
                # character head on local mentions
                c1 = ep.tile([128, 3, R], BF16)
                for co in range(3):
                    pc = eps.tile([128, R], F32, tag="pc", name=f"pc_{co}")
                    for ci in range(HC):
                        nc.tensor.matmul(
                            out=pc[:],
                            lhsT=wc1T_sb[:, ci, co * 128 : (co + 1) * 128],
                            rhs=rTl[:, ci, :],
                            start=(ci == 0),
                            stop=(ci == HC - 1),
                        )
                    nc.scalar.activation(
                        out=c1[:, co, :], in_=pc[:], func=AF.Relu,
                        bias=bc1c_sb[:, co : co + 1],
                    )
                plg = eps.tile([R, 18], F32, tag="lg")
                for co in range(3):
                    nc.tensor.matmul(
                        out=plg[:], lhsT=c1[:, co, :], rhs=wc2T_sb[:, co, :],
                        start=(co == 0), stop=False,
                    )
                nc.tensor.matmul(
                    out=plg[:], lhsT=one1[:], rhs=bc2r_sb[:], start=False, stop=True
                )
                cm = ep.tile([R, 1], F32)
                nc.vector.tensor_reduce(
                    out=cm[:], in_=plg[:], axis=mybir.AxisListType.X, op=OP.max
                )
                ncm = ep.tile([R, 1], F32)
                nc.vector.tensor_scalar_mul(ncm[:], cm[:], -1.0)
                cexp = ep.tile([R, 18], F32)
                cz = ep.tile([R, 1], F32)
                nc.scalar.activation(
                    out=cexp[:], in_=plg[:], func=AF.Exp, bias=ncm[:, 0:1],
                    accum_out=cz[:],
                )
                cscr = ep.tile([R, 18], F32)
                nc.vector.tensor_tensor(
                    out=cscr[:], in0=plg[:], in1=oneh_sb[:], op=OP.mult
                )
                sl = ep.tile([R, 1], F32)
                nc.vector.tensor_reduce(
                    out=sl[:], in_=cscr[:], axis=mybir.AxisListType.X, op=OP.add
                )
                lcz = ep.tile([R, 1], F32)
                nc.scalar.activation(out=lcz[:], in_=cz[:], func=AF.Ln)
                cev = ep.tile([R, 1], F32)
                nc.vector.tensor_tensor(
                    out=cev[:], in0=lcz[:], in1=cm[:], op=OP.add
                )
                nc.vector.tensor_tensor(
                    out=cev[:], in0=cev[:], in1=sl[:], op=OP.subtract
                )
                nc.tensor.matmul(
                    out=pl[:], lhsT=cev[:, 0:1], rhs=wch_sb[:], start=False,
                    stop=True,
                )
                lout = ep.tile([1, 1], F32)
                nc.vector.tensor_copy(out=lout[:], in_=pl[:])
                nc.sync.dma_start(out=loss.ap(), in_=lout[:])

    nc.compile()
    return nc


def _chunk_cols(w):
    """[K, O] -> [128, K//128, O]  (partition-chunked contraction dim)."""
    k, o = w.shape
    return np.ascontiguousarray(w.reshape(k // 128, 128, o).transpose(1, 0, 2))


def _chunk_vec(v, ncol):
    """[C] -> [128, ncol] column-chunks (zero padded)."""
    out = np.zeros((128, ncol), np.float32)
    for c in range(ncol):
        seg = v[c * 128 : (c + 1) * 128]
        out[: len(seg), c] = seg
    return out


def _prep_in_maps(inputs):
    bf = ml_dtypes.bfloat16

    seq = np.asarray(inputs["sequence_output"], np.float32).reshape(B * L, H)
    spk = np.asarray(inputs["speaker_emb"], np.float32)
    dummy = np.asarray(inputs["dummy_emb"], np.float32)
    gsrc = np.concatenate(
        [seq, spk, dummy, np.zeros((1, H), np.float32)], axis=0
    ).astype(bf)

    seg = np.asarray(inputs["mentions_seg"]).astype(np.int64)
    mstart = np.asarray(inputs["mention_start"]).astype(np.int64)
    mend = np.asarray(inputs["mention_end"]).astype(np.int64)
    sid = np.asarray(inputs["speaker_ids"]).astype(np.int64)[seg, mstart]
    gA = np.empty(N, np.int32)
    gB = np.empty(N, np.int32)
    gC = np.empty(N, np.int32)
    gA[0], gB[0], gC[0] = DUMMY_ROW, ZERO_ROW, ZERO_ROW
    gA[1:] = seg * L + mstart
    gB[1:] = seg * L + mend
    gC[1:] = B * L + sid
    g_all = np.stack([gA, gB, gC], axis=1)                       # [N, 3]
    gidx = np.ascontiguousarray(
        g_all.reshape(3, 128, 3).transpose(1, 0, 2)
    ).astype(np.int32)                                           # [128, 3, 3]

    W_pair1 = np.asarray(inputs["W_pair1"], np.float32)
    waT = _chunk_cols(np.ascontiguousarray(W_pair1[:, :H].T)).astype(bf)
    wbT = _chunk_cols(np.ascontiguousarray(W_pair1[:, H:].T)).astype(bf)
    w2T = _chunk_cols(
        np.ascontiguousarray(np.asarray(inputs["W_pair2"], np.float32).T)
    ).astype(bf)
    w3c = _chunk_vec(np.asarray(inputs["W_pair3"], np.float32)[0], 3).astype(bf)
    b1c = _chunk_vec(np.asarray(inputs["b_pair1"], np.float32), HC)
    b2c = _chunk_vec(np.asarray(inputs["b_pair2"], np.float32), 3)
    wm1T = _chunk_cols(
        np.ascontiguousarray(np.asarray(inputs["W_m1"], np.float32).T)
    ).astype(bf)
    bm1c = _chunk_vec(np.asarray(inputs["b_m1"], np.float32), 3)
    wm2T = _chunk_cols(
        np.ascontiguousarray(np.asarray(inputs["W_m2"], np.float32).T)
    ).astype(bf)
    bm2c = _chunk_vec(np.asarray(inputs["b_m2"], np.float32), 2)
    wm3c = _chunk_vec(np.asarray(inputs["W_m3"], np.float32)[0], 2).astype(bf)
    wc1T = _chunk_cols(
        np.ascontiguousarray(np.asarray(inputs["W_c1"], np.float32).T)
    ).astype(bf)
    bc1c = _chunk_vec(np.asarray(inputs["b_c1"], np.float32), 3)
    wc2T = _chunk_cols(
        np.ascontiguousarray(np.asarray(inputs["W_c2"], np.float32).T)
    ).astype(bf)
    bc2r = np.asarray(inputs["b_c2"], np.float32).reshape(1, 18)

    link_first = np.asarray(inputs["link_first"]).astype(np.int64)
    link_second = np.asarray(inputs["link_second"]).astype(np.int64)
    label = np.asarray(inputs["character_label"]).astype(np.int64)

    mult = np.zeros((N, N), np.float32)
    np.add.at(mult, (link_second, link_first), 1.0)
    has_link = mult.sum(axis=1) > 0
    wnll_full = ((np.arange(N) >= 1) & has_link).astype(np.float32)
    mult[~has_link, 0] = 1.0  # keep log(E) finite; weight is 0 there

    mask_full = np.where(
        np.arange(N)[None, :] >= np.arange(N)[:, None], np.float32(NEG), 0.0
    ).astype(np.float32)

    oneh_full = np.zeros((N, 18), np.float32)
    wch_full = np.zeros(N, np.float32)
    oneh_full[np.arange(1, N), label] = 1.0
    wch_full[1:] = 1.0

    ident = np.eye(128, dtype=bf)

    shared = dict(
        gsrc=gsrc, gidx=gidx, ident=ident,
        waT=waT, wbT=wbT, w2T=w2T, w3c=w3c, b1c=b1c, b2c=b2c,
        wm1T=wm1T, bm1c=bm1c, wm2T=wm2T, bm2c=bm2c, wm3c=wm3c,
        wc1T=wc1T, bc1c=bc1c, wc2T=wc2T, bc2r=bc2r,
    )
    in_maps = []
    for d in range(NC_):
        rows = slice(d * R, (d + 1) * R)
        m = dict(shared)
        m["gidxl"] = np.ascontiguousarray(g_all[rows]).astype(np.int32)
        m["maskb"] = np.ascontiguousarray(mask_full[rows])
        m["multb"] = np.ascontiguousarray(mult[rows])
        m["wnll"] = np.ascontiguousarray(wnll_full[rows]).reshape(R, 1)
        m["oneh"] = np.ascontiguousarray(oneh_full[rows])
        m["wch"] = np.ascontiguousarray(wch_full[rows]).reshape(R, 1)
        in_maps.append(m)
    return in_maps


def kernel(**inputs):
    global LAST_RESULT
    in_maps = _prep_in_maps(inputs)

    if "nc" not in _CACHE:
        _CACHE["nc"] = _build_program()
    nc = _CACHE["nc"]

    res = run_bass_kernel_spmd(nc, in_maps, core_ids=list(range(NC_)))
    LAST_RESULT = res
    total = np.float32(0.0)
    for d in range(NC_):
        total += np.float32(res.results[d]["loss"][0, 0])
    return np.asarray(total, dtype=np.float32)


if __name__ == "__main__":
    import reference

    inputs = {k: np.asarray(v) for k, v in reference.setup_inputs().items()}
    out = kernel(**inputs)
    print("kernel out:", out)



# revision 3
# speedup vs baseline: 1.9212x; 1.9212x over previous
"""Trainium2 Bass kernel for nn_JointLearningModel (coref-style joint model).

v2 redesign vs baseline:
- Triangular pair grid: row i only needs columns j <= i (the causal mask
  kills the rest) -> ~51% of the pairwise MLP work.
- Block-cyclic row sharding (core d owns grid rows d, d+8, ..., d+376) so
  every core gets an identical balanced triangular profile and the SAME
  compiled program (column extents are compile-time constants).
- Rows batched (8/4/2/1 rows per matmul batch, padded to <=512 psum cols)
  to amortize matmul/LDWEIGHTS/relu instruction overheads on short rows.
- h2 GEMM in fp8e4 DoubleRow perf mode (2 k-chunks per matmul) with host
  weight scaling: Wa,Wb,b1 x16; W2 x64 (fp8); w3 /1024 compensates.
- h1 = relu(A_j + B_r) split across DVE (tensor_scalar, 2x_2p) and GPSIMD;
  h2 relu+bias split DVE/ACT; pair-score rows evacuated via ACT.

`repeat` builds a program that runs the whole computation N times —
used only for slope-based timing (exec = dT/dN), not for grading runs.
"""

import numpy as np
import ml_dtypes

import concourse.bass as bass
import concourse.mybir as mybir
import concourse.tile as tile
from concourse import bacc
from concourse.bass_utils import run_bass_kernel_spmd

F32 = mybir.dt.float32
BF16 = mybir.dt.bfloat16
FP8 = mybir.dt.float8e4
I32 = mybir.dt.int32
AF = mybir.ActivationFunctionType
OP = mybir.AluOpType
DR = mybir.MatmulPerfMode.DoubleRow

B, L, H, M = 8, 512, 768, 383
N = M + 1          # 384 rows/cols of the pair grid
NC_ = 8            # cores
R = N // NC_       # 48 rows per core (block-cyclic: core d has rows d::8)
HC = H // 128      # 6 k-chunks of the hidden dim
NEG = -10000.0
NSRC = B * L + 400 + 1 + 1   # seq rows + speaker rows + dummy + zeros row
DUMMY_ROW = B * L + 400
ZERO_ROW = DUMMY_ROW + 1

USE_FP8 = True
S1 = 16.0          # scale on Wa/Wb/b1 -> h1
S2 = 64.0          # scale on W2 (fp8 path only)
MAXNB = 512        # max padded batch width (one psum bank of f32)

_CACHE = {}
LAST_RESULT = None


def _batches():
    """(k0, nb, cpad): rows k0..k0+nb-1 at padded col extent cpad."""
    out = []
    k = 0
    while k < R:
        if k < 8:
            nb = 8
        elif k < 16:
            nb = 4
        elif k < 32:
            nb = 2
        else:
            nb = 1
        clast = 8 * (k + nb)           # max col extent in the batch
        cpad = (clast + 15) // 16 * 16
        assert nb * cpad <= MAXNB
        out.append((k, nb, cpad))
        k += nb
    return out


BATCHES = _batches()
SBLKF_LEN = sum(nb * cp_ for _, nb, cp_ in BATCHES)


def _build_program(repeat=1, h1_eng="split", use_fp8=None, h1_once=False, skip_sr=False):
    if use_fp8 is None:
        use_fp8 = USE_FP8
    nc = bacc.Bacc(
        "TRN2", target_bir_lowering=False, debug=False, enable_asserts=False
    )

    def din(name, shape, dt):
        return nc.dram_tensor(name, list(shape), dt, kind="ExternalInput")

    w2dt = FP8 if use_fp8 else BF16

    # gather source + offset tables
    gsrc = din("gsrc", [NSRC, H], BF16)
    gidx = din("gidx", [128, 3, 3], I32)      # [p, tile, {start,end,spk}]
    gidxl = din("gidxl", [R, 3], I32)         # local rows (per-core)
    ident = din("ident", [128, 128], BF16)
    # pair MLP weights
    waT = din("waT", [128, HC, H], FP8)      # waT[p,ci,o] = S1*Wa.T[ci*128+p, o]
    wbT = din("wbT", [128, HC, H], FP8)
    w2T = din("w2T", [128, HC, H // 2], w2dt)
    w3c = din("w3c", [128, 3], BF16)
    w3f8 = din("w3f8", [128, 2, 16], FP8)
    b2r = din("b2r", [128, 3], F32)
    b1c = din("b1c", [128, HC], F32)
    b2c = din("b2c", [128, 3], F32)
    # mention-score MLP
    wm1T = din("wm1T", [128, HC, H // 2], FP8)
    bm1c = din("bm1c", [128, 3], F32)
    wm2T = din("wm2T", [128, 3, H // 4], BF16)
    bm2c = din("bm2c", [128, 2], F32)
    wm3c = din("wm3c", [128, 2], BF16)
    # character head
    wc1T = din("wc1T", [128, HC, H // 2], FP8)
    bc1c = din("bc1c", [128, 3], F32)
    wc2T = din("wc2T", [128, 3, 18], BF16)
    bc2r = din("bc2r", [1, 18], F32)
    # per-core loss plumbing
    maskb = din("maskb", [R, N], F32)
    multb = din("multb", [R, N], F32)
    wnll = din("wnll", [R, 1], F32)
    oneh = din("oneh", [R, 18], F32)
    wch = din("wch", [R, 1], F32)

    loss = nc.dram_tensor("loss", [1, 1], F32, kind="ExternalOutput")

    with tile.TileContext(nc) as tc:
        with tc.tile_pool(name="const", bufs=1) as cp:
            def load(name, h):
                t = cp.tile(list(h.shape), h.dtype, name=name)
                nc.sync.dma_start(out=t[:], in_=h.ap())
                return t

            ident_sb = load("ident_sb", ident)
            gidx_sb = load("gidx_sb", gidx)
            gidxl_sb = load("gidxl_sb", gidxl)
            waT_sb = load("waT_sb", waT)
            wbT_sb = load("wbT_sb", wbT)
            w2T_sb = load("w2T_sb", w2T)
            w3c_sb = load("w3c_sb", w3c)
            b1c_sb = load("b1c_sb", b1c)
            b2c_sb = load("b2c_sb", b2c)
            w3f8_sb = load("w3f8_sb", w3f8)
            b2r_sb = load("b2r_sb", b2r)
            wm1T_sb = load("wm1T_sb", wm1T)
            bm1c_sb = load("bm1c_sb", bm1c)
            wm2T_sb = load("wm2T_sb", wm2T)
            bm2c_sb = load("bm2c_sb", bm2c)
            wm3c_sb = load("wm3c_sb", wm3c)
            wc1T_sb = load("wc1T_sb", wc1T)
            bc1c_sb = load("bc1c_sb", bc1c)
            wc2T_sb = load("wc2T_sb", wc2T)
            bc2r_sb = load("bc2r_sb", bc2r)
            maskb_sb = load("maskb_sb", maskb)
            multb_sb = load("multb_sb", multb)
            wnll_sb = load("wnll_sb", wnll)
            oneh_sb = load("oneh_sb", oneh)
            wch_sb = load("wch_sb", wch)

            one1 = cp.tile([1, R], F32)
            nc.vector.memset(one1[:], 1.0)

            def body(rep):
                def ct(shape, dt, tag, bufs=1):
                    return cp.tile(
                        shape, dt, tag=tag, bufs=bufs, name=f"{tag}_{rep}"
                    )

                at_sb = ct([128, HC, N], BF16, "at_sb")    # S1*A.T
                bb_sb = ct([128, HC, R], F32, "bb_sb")     # S1*(Bm.T+b1)
                rT = ct([128, HC, N], FP8, "rT")          # all_reps.T
                rTl = ct([128, HC, R], FP8, "rTl")        # local reps.T
                mskms = ct([R, N], F32, "mskms")           # mask + ms[j]
                sblk = ct([R, N], F32, "sblk")             # scores, row layout
                sblkf = ct([1, SBLKF_LEN], F32, "sblkf")
                nc.vector.memset(sblk[:], 0.0)

                # ---------- gather mention representations ----------
                reps = []
                for t in range(3):
                    g3 = []
                    for s in range(3):
                        g = ct([128, H], BF16, f"g{s}", bufs=2)
                        nc.gpsimd.indirect_dma_start(
                            out=g[:],
                            out_offset=None,
                            in_=gsrc.ap(),
                            in_offset=bass.IndirectOffsetOnAxis(
                                ap=gidx_sb[:, t, s : s + 1], axis=0
                            ),
                        )
                        g3.append(g)
                    rep_t = ct([128, H], BF16, f"rep{t}")
                    nc.vector.tensor_tensor(
                        out=rep_t[:], in0=g3[0][:], in1=g3[1][:], op=OP.add
                    )
                    nc.vector.tensor_tensor(
                        out=rep_t[:], in0=rep_t[:], in1=g3[2][:], op=OP.add
                    )
                    reps.append(rep_t)
                gl3 = []
                for s in range(3):
                    gl = ct([R, H], BF16, f"gl{s}")
                    nc.gpsimd.indirect_dma_start(
                        out=gl[:],
                        out_offset=None,
                        in_=gsrc.ap(),
                        in_offset=bass.IndirectOffsetOnAxis(
                            ap=gidxl_sb[:, s : s + 1], axis=0
                        ),
                    )
                    gl3.append(gl)
                repl = ct([R, H], BF16, "repl")
                nc.vector.tensor_tensor(
                    out=repl[:], in0=gl3[0][:], in1=gl3[1][:], op=OP.add
                )
                nc.vector.tensor_tensor(
                    out=repl[:], in0=repl[:], in1=gl3[2][:], op=OP.add
                )

                # ---------- transpose to [H, mention] layout ----------
                with tc.tile_pool(
                    name=f"tp_ps{rep}", bufs=2, space="PSUM"
                ) as tpp:
                    for c in range(HC):
                        pt = tpp.tile([128, N], BF16, tag="tp", name=f"pt_{c}")
                        for t in range(3):
                            nc.tensor.transpose(
                                out=pt[:, t * 128 : (t + 1) * 128],
                                in_=reps[t][:, c * 128 : (c + 1) * 128],
                                identity=ident_sb[:],
                            )
                        nc.vector.tensor_copy(out=rT[:, c, :], in_=pt[:])
                    for c in range(HC):
                        ptl = tpp.tile([128, R], BF16, tag="tpl", name=f"ptl_{c}")
                        nc.tensor.transpose(
                            out=ptl[:],
                            in_=repl[:, c * 128 : (c + 1) * 128],
                            identity=ident_sb[:R, :R],
                        )
                        nc.scalar.copy(out=rTl[:, c, :], in_=ptl[:])

                # ---------- preamble matmuls: A.T, Bb, ms, mask+ms ----------
                with tc.tile_pool(
                    name=f"pre_ps{rep}", bufs=2, space="PSUM"
                ) as pp:
                    for co in range(HC):
                        pa = pp.tile([128, N], F32, tag="big", name=f"pa_{co}")
                        for a in range(HC // 2):
                            nc.tensor.matmul(
                                out=pa[:],
                                lhsT=waT_sb[
                                    :, 2 * a : 2 * a + 2,
                                    co * 128 : (co + 1) * 128,
                                ],
                                rhs=rT[:, 2 * a : 2 * a + 2, :],
                                start=(a == 0),
                                stop=(a == HC // 2 - 1),
                                perf_mode=DR,
                            )
                        if co % 2 == 0:
                            nc.scalar.copy(out=at_sb[:, co, :], in_=pa[:])
                        else:
                            nc.vector.tensor_copy(out=at_sb[:, co, :], in_=pa[:])
                    for co in range(HC):
                        pb = pp.tile([128, R], F32, tag="small", name=f"pb_{co}")
                        for a in range(HC // 2):
                            nc.tensor.matmul(
                                out=pb[:],
                                lhsT=wbT_sb[
                                    :, 2 * a : 2 * a + 2,
                                    co * 128 : (co + 1) * 128,
                                ],
                                rhs=rTl[:, 2 * a : 2 * a + 2, :],
                                start=(a == 0),
                                stop=(a == HC // 2 - 1),
                                perf_mode=DR,
                            )
                        nc.vector.tensor_scalar(
                            out=bb_sb[:, co, :],
                            in0=pb[:],
                            scalar1=b1c_sb[:, co : co + 1],
                            scalar2=None,
                            op0=OP.add,
                        )
                    # mention score MLP (768 -> 384 -> 192 -> 1)
                    ms1 = ct([128, 3, N], BF16, "ms1")
                    for co in range(3):
                        pm = pp.tile([128, N], F32, tag="big", name=f"pm_{co}")
                        for a in range(HC // 2):
                            nc.tensor.matmul(
                                out=pm[:],
                                lhsT=wm1T_sb[
                                    :, 2 * a : 2 * a + 2,
                                    co * 128 : (co + 1) * 128,
                                ],
                                rhs=rT[:, 2 * a : 2 * a + 2, :],
                                start=(a == 0),
                                stop=(a == HC // 2 - 1),
                                perf_mode=DR,
                            )
                        nc.scalar.activation(
                            out=ms1[:, co, :],
                            in_=pm[:],
                            func=AF.Relu,
                            bias=bm1c_sb[:, co : co + 1],
                        )
                    ms2 = ct([128, 2, N], BF16, "ms2")
                    for co, sz in enumerate((128, 64)):
                        pm2 = pp.tile([128, N], F32, tag="big", name=f"pm2_{co}")
                        for ci in range(3):
                            nc.tensor.matmul(
                                out=pm2[:sz, :],
                                lhsT=wm2T_sb[:, ci, co * 128 : co * 128 + sz],
                                rhs=ms1[:, ci, :],
                                start=(ci == 0),
                                stop=(ci == 2),
                            )
                        nc.scalar.activation(
                            out=ms2[:sz, co, :],
                            in_=pm2[:sz, :],
                            func=AF.Relu,
                            bias=bm2c_sb[:sz, co : co + 1],
                        )
                    pms = pp.tile([1, N], F32, tag="small")
                    nc.tensor.matmul(
                        out=pms[:], lhsT=wm3c_sb[:, 0:1], rhs=ms2[:, 0, :],
                        start=True, stop=False,
                    )
                    nc.tensor.matmul(
                        out=pms[:], lhsT=wm3c_sb[:64, 1:2], rhs=ms2[:64, 1, :],
                        start=False, stop=True,
                    )
                    ms_sb = ct([1, N], F32, "ms_sb")
                    nc.vector.tensor_copy(out=ms_sb[:], in_=pms[:])
                    pbc = pp.tile([R, N], F32, tag="big")
                    nc.tensor.matmul(
                        out=pbc[:], lhsT=one1[:], rhs=ms_sb[:],
                        start=True, stop=True,
                    )
                    nc.vector.tensor_tensor(
                        out=mskms[:], in0=pbc[:], in1=maskb_sb[:], op=OP.add
                    )

                # ---------- main loop over row batches ----------
                with (
                    tc.tile_pool(name=f"h2_ps{rep}", bufs=1, space="PSUM") as hps,
                    tc.tile_pool(name=f"sr_ps{rep}", bufs=1, space="PSUM") as sps,
                ):
                    off = 0
                    for bi, (k0, nb, cpad) in enumerate(BATCHES):
                        nbc = nb * cpad
                        h1t = ct([128, HC, MAXNB], w2dt, "h1", bufs=4)
                        if nb >= 2:
                            tmp = ct([128, HC, MAXNB], BF16, "h1tmp", bufs=2)
                            for ci in range(HC):
                                eng = nc.vector if ci < 3 else nc.gpsimd
                                o3 = tmp[:, ci, :nbc].rearrange(
                                    "p (q j) -> p q j", q=nb
                                )
                                eng.tensor_tensor(
                                    out=o3,
                                    in0=at_sb[:, ci, :cpad]
                                    .unsqueeze(1)
                                    .broadcast_to([128, nb, cpad]),
                                    in1=bb_sb[:, ci, k0 : k0 + nb]
                                    .unsqueeze(2)
                                    .broadcast_to([128, nb, cpad]),
                                    op=OP.add,
                                )
                            if bi % 2 == 0:
                                nc.vector.tensor_scalar(
                                    out=h1t[:, :, :nbc],
                                    in0=tmp[:, :, :nbc],
                                    scalar1=0.0,
                                    scalar2=None,
                                    op0=OP.max,
                                )
                            else:
                                nc.scalar.activation(
                                    out=h1t[:, :, :nbc],
                                    in_=tmp[:, :, :nbc],
                                    func=AF.Relu,
                                )
                        else:
                            k = k0
                            for ci in range(HC):
                                seg = h1t[:, ci, 0:cpad]
                                if ci < 3:
                                    nc.vector.tensor_scalar(
                                        out=seg,
                                        in0=at_sb[:, ci, :cpad],
                                        scalar1=bb_sb[:, ci, k : k + 1],
                                        scalar2=0.0,
                                        op0=OP.add,
                                        op1=OP.max,
                                    )
                                else:
                                    nc.gpsimd.tensor_scalar(
                                        out=seg,
                                        in0=at_sb[:, ci, :cpad],
                                        scalar1=bb_sb[:, ci, k : k + 1],
                                        scalar2=0.0,
                                        op0=OP.add,
                                        op1=OP.max,
                                    )
                        h2sb = ct([128, 1, MAXNB], BF16, "h2sb", bufs=3)
                        h2s8 = ct([128, 2, MAXNB], FP8, "h2s8", bufs=3)
                        for co in range(3):
                            ph = hps.tile(
                                [128, MAXNB], F32, tag=f"h2_{co}",
                                name=f"ph_{bi}_{co}", bufs=2,
                            )
                            if use_fp8:
                                for a in range(HC // 2):
                                    nc.tensor.matmul(
                                        out=ph[:, :nbc],
                                        lhsT=w2T_sb[
                                            :, 2 * a : 2 * a + 2,
                                            co * 128 : (co + 1) * 128,
                                        ],
                                        rhs=h1t[:, 2 * a : 2 * a + 2, :nbc],
                                        start=(a == 0),
                                        stop=(a == HC // 2 - 1),
                                        perf_mode=DR,
                                    )
                            else:
                                for ci in range(HC):
                                    nc.tensor.matmul(
                                        out=ph[:, :nbc],
                                        lhsT=w2T_sb[
                                            :, ci, co * 128 : (co + 1) * 128
                                        ],
                                        rhs=h1t[:, ci, :nbc],
                                        start=(ci == 0),
                                        stop=(ci == HC - 1),
                                    )
                            if co == 0:
                                nc.vector.tensor_scalar(
                                    out=h2sb[:, 0, :nbc],
                                    in0=ph[:, :nbc],
                                    scalar1=b2c_sb[:, co : co + 1],
                                    scalar2=0.0,
                                    op0=OP.add,
                                    op1=OP.max,
                                )
                            else:
                                nc.scalar.activation(
                                    out=h2s8[:, co - 1, :nbc],
                                    in_=ph[:, :nbc],
                                    func=AF.Relu,
                                    bias=b2r_sb[:, co : co + 1],
                                    scale=1.0 / (S1 * S2),
                                )
                        if skip_sr:
                            off += nbc
                            continue
                        srp = sps.tile(
                            [1, MAXNB], F32, tag="sr", name=f"sr_{bi}", bufs=2
                        )
                        nc.tensor.matmul(
                            out=srp[:, :nbc],
                            lhsT=w3c_sb[:, 0:1],
                            rhs=h2sb[:, 0, :nbc],
                            start=True,
                            stop=False,
                        )
                        nc.tensor.matmul(
                            out=srp[:, :nbc],
                            lhsT=w3f8_sb[:, :, 0:1],
                            rhs=h2s8[:, :, :nbc],
                            start=False,
                            stop=True,
                            perf_mode=DR,
                        )
                        nc.scalar.copy(
                            out=sblkf[:, off : off + nbc], in_=srp[:, :nbc]
                        )
                        nc.gpsimd.dma_start(
                            out=sblk[k0 : k0 + nb, 0:cpad],
                            in_=sblkf[:, off : off + nbc],
                        )
                        off += nbc

                # ---------- epilogue: row-softmax loss + char CE ----------
                with tc.tile_pool(
                    name=f"ep_ps{rep}", bufs=2, space="PSUM"
                ) as eps:
                    x = ct([R, N], F32, "x")
                    nc.vector.tensor_tensor(
                        out=x[:], in0=sblk[:], in1=mskms[:], op=OP.add
                    )
                    rm = ct([R, 1], F32, "rm")
                    nc.vector.tensor_reduce(
                        out=rm[:], in_=x[:], axis=mybir.AxisListType.X, op=OP.max
                    )
                    nrm = ct([R, 1], F32, "nrm")
                    nc.vector.tensor_scalar_mul(nrm[:], rm[:], -1.0)
                    pexp = ct([R, N], F32, "pexp")
                    z = ct([R, 1], F32, "z")
                    nc.scalar.activation(
                        out=pexp[:], in_=x[:], func=AF.Exp, bias=nrm[:, 0:1],
                        accum_out=z[:],
                    )
                    escr = ct([R, N], F32, "escr")
                    nc.vector.tensor_tensor(
                        out=escr[:], in0=pexp[:], in1=multb_sb[:], op=OP.mult
                    )
                    e = ct([R, 1], F32, "e")
                    nc.vector.tensor_reduce(
                        out=e[:], in_=escr[:], axis=mybir.AxisListType.X,
                        op=OP.add,
                    )
                    lz = ct([R, 1], F32, "lz")
                    nc.scalar.activation(out=lz[:], in_=z[:], func=AF.Ln)
                    le = ct([R, 1], F32, "le")
                    nc.scalar.activation(out=le[:], in_=e[:], func=AF.Ln)
                    tnll = ct([R, 1], F32, "tnll")
                    nc.vector.tensor_tensor(
                        out=tnll[:], in0=lz[:], in1=le[:], op=OP.subtract
                    )
                    pl = eps.tile([1, 1], F32, tag="loss", bufs=1)
                    nc.tensor.matmul(
                        out=pl[:], lhsT=tnll[:, 0:1], rhs=wnll_sb[:],
                        start=True, stop=False,
                    )
                    # character head on local mentions
                    c1 = ct([128, 3, R], BF16, "c1")
                    for co in range(3):
                        pc = eps.tile([128, R], F32, tag="pc", name=f"pc_{co}")
                        for a in range(HC // 2):
                            nc.tensor.matmul(
                                out=pc[:],
                                lhsT=wc1T_sb[
                                    :, 2 * a : 2 * a + 2,
                                    co * 128 : (co + 1) * 128,
                                ],
                                rhs=rTl[:, 2 * a : 2 * a + 2, :],
                                start=(a == 0),
                                stop=(a == HC // 2 - 1),
                                perf_mode=DR,
                            )
                        nc.scalar.activation(
                            out=c1[:, co, :], in_=pc[:], func=AF.Relu,
                            bias=bc1c_sb[:, co : co + 1],
                        )
                    plg = eps.tile([R, 18], F32, tag="lg")
                    for co in range(3):
                        nc.tensor.matmul(
                            out=plg[:], lhsT=c1[:, co, :], rhs=wc2T_sb[:, co, :],
                            start=(co == 0), stop=False,
                        )
                    nc.tensor.matmul(
                        out=plg[:], lhsT=one1[:], rhs=bc2r_sb[:],
                        start=False, stop=True,
                    )
                    cm = ct([R, 1], F32, "cm")
                    nc.vector.tensor_reduce(
                        out=cm[:], in_=plg[:], axis=mybir.AxisListType.X,
                        op=OP.max,
                    )
                    ncm = ct([R, 1], F32, "ncm")
                    nc.vector.tensor_scalar_mul(ncm[:], cm[:], -1.0)
                    cexp = ct([R, 18], F32, "cexp")
                    cz = ct([R, 1], F32, "cz")
                    nc.scalar.activation(
                        out=cexp[:], in_=plg[:], func=AF.Exp, bias=ncm[:, 0:1],
                        accum_out=cz[:],
                    )
                    cscr = ct([R, 18], F32, "cscr")
                    nc.vector.tensor_tensor(
                        out=cscr[:], in0=plg[:], in1=oneh_sb[:], op=OP.mult
                    )
                    sl = ct([R, 1], F32, "sl")
                    nc.vector.tensor_reduce(
                        out=sl[:], in_=cscr[:], axis=mybir.AxisListType.X,
                        op=OP.add,
                    )
                    lcz = ct([R, 1], F32, "lcz")
                    nc.scalar.activation(out=lcz[:], in_=cz[:], func=AF.Ln)
                    cev = ct([R, 1], F32, "cev")
                    nc.vector.tensor_tensor(
                        out=cev[:], in0=lcz[:], in1=cm[:], op=OP.add
                    )
                    nc.vector.tensor_tensor(
                        out=cev[:], in0=cev[:], in1=sl[:], op=OP.subtract
                    )
                    nc.tensor.matmul(
                        out=pl[:], lhsT=cev[:, 0:1], rhs=wch_sb[:],
                        start=False, stop=True,
                    )
                    lout = ct([1, 1], F32, "lout")
                    nc.vector.tensor_copy(out=lout[:], in_=pl[:])
                    nc.sync.dma_start(out=loss.ap(), in_=lout[:])

            for rep in range(repeat):
                body(rep)

    nc.compile()
    return nc


def _chunk_cols(w):
    """[K, O] -> [128, K//128, O]  (partition-chunked contraction dim)."""
    k, o = w.shape
    return np.ascontiguousarray(w.reshape(k // 128, 128, o).transpose(1, 0, 2))


def _chunk_vec(v, ncol):
    """[C] -> [128, ncol] column-chunks (zero padded)."""
    out = np.zeros((128, ncol), np.float32)
    for c in range(ncol):
        seg = v[c * 128 : (c + 1) * 128]
        out[: len(seg), c] = seg
    return out


def _prep_in_maps(inputs):
    bf = ml_dtypes.bfloat16
    f8 = ml_dtypes.float8_e4m3

    seq = np.asarray(inputs["sequence_output"], np.float32).reshape(B * L, H)
    spk = np.asarray(inputs["speaker_emb"], np.float32)
    dummy = np.asarray(inputs["dummy_emb"], np.float32)
    gsrc = np.concatenate(
        [seq, spk, dummy, np.zeros((1, H), np.float32)], axis=0
    ).astype(bf)

    seg = np.asarray(inputs["mentions_seg"]).astype(np.int64)
    mstart = np.asarray(inputs["mention_start"]).astype(np.int64)
    mend = np.asarray(inputs["mention_end"]).astype(np.int64)
    sid = np.asarray(inputs["speaker_ids"]).astype(np.int64)[seg, mstart]
    gA = np.empty(N, np.int32)
    gB = np.empty(N, np.int32)
    gC = np.empty(N, np.int32)
    gA[0], gB[0], gC[0] = DUMMY_ROW, ZERO_ROW, ZERO_ROW
    gA[1:] = seg * L + mstart
    gB[1:] = seg * L + mend
    gC[1:] = B * L + sid
    g_all = np.stack([gA, gB, gC], axis=1)                       # [N, 3]
    gidx = np.ascontiguousarray(
        g_all.reshape(3, 128, 3).transpose(1, 0, 2)
    ).astype(np.int32)                                           # [128, 3, 3]

    W_pair1 = np.asarray(inputs["W_pair1"], np.float32)
    waT = _chunk_cols(np.ascontiguousarray(W_pair1[:, :H].T) * S1).astype(f8)
    wbT = _chunk_cols(np.ascontiguousarray(W_pair1[:, H:].T) * S1).astype(f8)
    w2 = np.ascontiguousarray(np.asarray(inputs["W_pair2"], np.float32).T)
    if USE_FP8:
        w2T = _chunk_cols(w2 * S2).astype(f8)
        sc = S1 * S2
    else:
        w2T = _chunk_cols(w2).astype(bf)
        sc = S1
    w3c = _chunk_vec(
        np.asarray(inputs["W_pair3"], np.float32)[0] / sc, 3
    ).astype(bf)
    w3raw = np.asarray(inputs["W_pair3"], np.float32)[0]
    w3f8 = np.zeros((128, 2, 16), np.float32)
    w3f8[:, 0, 0] = w3raw[128:256]
    w3f8[:, 1, 0] = w3raw[256:384]
    w3f8 = w3f8.astype(f8)
    b2r = _chunk_vec(np.asarray(inputs["b_pair2"], np.float32), 3)
    b1c = _chunk_vec(np.asarray(inputs["b_pair1"], np.float32) * S1, HC)
    b2c = _chunk_vec(np.asarray(inputs["b_pair2"], np.float32) * sc, 3)
    wm1T = _chunk_cols(
        np.ascontiguousarray(np.asarray(inputs["W_m1"], np.float32).T)
    ).astype(f8)
    bm1c = _chunk_vec(np.asarray(inputs["b_m1"], np.float32), 3)
    wm2T = _chunk_cols(
        np.ascontiguousarray(np.asarray(inputs["W_m2"], np.float32).T)
    ).astype(bf)
    bm2c = _chunk_vec(np.asarray(inputs["b_m2"], np.float32), 2)
    wm3c = _chunk_vec(np.asarray(inputs["W_m3"], np.float32)[0], 2).astype(bf)
    wc1T = _chunk_cols(
        np.ascontiguousarray(np.asarray(inputs["W_c1"], np.float32).T)
    ).astype(f8)
    bc1c = _chunk_vec(np.asarray(inputs["b_c1"], np.float32), 3)
    wc2T = _chunk_cols(
        np.ascontiguousarray(np.asarray(inputs["W_c2"], np.float32).T)
    ).astype(bf)
    bc2r = np.asarray(inputs["b_c2"], np.float32).reshape(1, 18)

    link_first = np.asarray(inputs["link_first"]).astype(np.int64)
    link_second = np.asarray(inputs["link_second"]).astype(np.int64)
    label = np.asarray(inputs["character_label"]).astype(np.int64)

    mult = np.zeros((N, N), np.float32)
    np.add.at(mult, (link_second, link_first), 1.0)
    has_link = mult.sum(axis=1) > 0
    wnll_full = ((np.arange(N) >= 1) & has_link).astype(np.float32)
    mult[~has_link, 0] = 1.0  # keep log(E) finite; weight is 0 there

    mask_full = np.where(
        np.arange(N)[None, :] >= np.arange(N)[:, None], np.float32(NEG), 0.0
    ).astype(np.float32)

    oneh_full = np.zeros((N, 18), np.float32)
    wch_full = np.zeros(N, np.float32)
    oneh_full[np.arange(1, N), label] = 1.0
    wch_full[1:] = 1.0

    ident = np.eye(128, dtype=bf)

    shared = dict(
        gsrc=gsrc, gidx=gidx, ident=ident,
        waT=waT, wbT=wbT, w2T=w2T, w3c=w3c, b1c=b1c, b2c=b2c,
        w3f8=w3f8, b2r=b2r,
        wm1T=wm1T, bm1c=bm1c, wm2T=wm2T, bm2c=bm2c, wm3c=wm3c,
        wc1T=wc1T, bc1c=bc1c, wc2T=wc2T, bc2r=bc2r,
    )
    in_maps = []
    for d in range(NC_):
        rows = np.arange(d, N, NC_)          # block-cyclic row set
        m = dict(shared)
        m["gidxl"] = np.ascontiguousarray(g_all[rows]).astype(np.int32)
        m["maskb"] = np.ascontiguousarray(mask_full[rows])
        m["multb"] = np.ascontiguousarray(mult[rows])
        m["wnll"] = np.ascontiguousarray(wnll_full[rows]).reshape(R, 1)
        m["oneh"] = np.ascontiguousarray(oneh_full[rows])
        m["wch"] = np.ascontiguousarray(wch_full[rows]).reshape(R, 1)
        in_maps.append(m)
    return in_maps


def kernel(**inputs):
    global LAST_RESULT
    in_maps = _prep_in_maps(inputs)

    if "nc" not in _CACHE:
        _CACHE["nc"] = _build_program()
    nc = _CACHE["nc"]

    res = run_bass_kernel_spmd(nc, in_maps, core_ids=list(range(NC_)))
    LAST_RESULT = res
    total = np.float32(0.0)
    for d in range(NC_):
        total += np.float32(res.results[d]["loss"][0, 0])
    return np.asarray(total, dtype=np.float32)


if __name__ == "__main__":
    import reference

    inputs = {k: np.asarray(v) for k, v in reference.setup_inputs().items()}
    out = kernel(**inputs)
    print("kernel out:", out)
